# revision 1
# baseline (speedup 1.0000x reference)
"""ECE (confidence calibration) kernel for 8 Trainium2 NeuronCores.

Strategy (data-parallel, per sharding hint): split N=2^24 samples across 8
cores. Each core computes, for thresholds k=0..15, the cumulative weighted
histogram D_k = sum_i (c_i - a_i) * [15*c_i <= k] via fused
scalar_tensor_tensor (compare+mult+reduce in one DVE pass per threshold).
Host sums the per-core partials and finishes:
    ece = (1/N) * sum_b |D_{b+1} - D_b|
Elements with c <= 0 land in every D_k and cancel in the differences;
elements with c > 1 land in no D_k — both match the reference's
valid-mask/overflow-bin semantics exactly.
"""
import numpy as np
import concourse.bass as bass
import concourse.bacc as bacc
import concourse.mybir as mybir
from concourse.tile import TileContext
from concourse.bass_utils import run_bass_kernel_spmd

N = 16777216
NUM_BINS = 15
N_CORES = 8
P = 128
FD = 2048
M = N // N_CORES  # per-core elements
N_TILES = M // (P * FD)
NK = NUM_BINS + 1  # thresholds k = 0..15


def build_nc():
    nc = bacc.Bacc(None)
    conf = nc.dram_tensor("confidences", [M], mybir.dt.float32, kind="ExternalInput")
    acc_in = nc.dram_tensor("accuracies", [M], mybir.dt.float32, kind="ExternalInput")
    out = nc.dram_tensor(
        "partials", [P, N_TILES * NK], mybir.dt.float32, kind="ExternalOutput"
    )
    conf_t = conf.rearrange("(n p f) -> n p f", p=P, f=FD)
    acc_t = acc_in.rearrange("(n p f) -> n p f", p=P, f=FD)

    with TileContext(nc) as tc:
        with (
            tc.tile_pool(name="io", bufs=3) as io_pool,
            tc.tile_pool(name="work", bufs=2) as work_pool,
            tc.tile_pool(name="accp", bufs=1) as acc_pool,
        ):
            acc_sb = acc_pool.tile([P, N_TILES * NK], mybir.dt.float32)
            for j in range(N_TILES):
                c_tile = io_pool.tile([P, FD], mybir.dt.float32, tag="c")
                a_tile = io_pool.tile([P, FD], mybir.dt.float32, tag="a")
                nc.sync.dma_start(out=c_tile[:, :], in_=conf_t[j, :, :])
                nc.sync.dma_start(out=a_tile[:, :], in_=acc_t[j, :, :])
                d_tile = work_pool.tile([P, FD], mybir.dt.float32, tag="d")
                x_tile = work_pool.tile([P, FD], mybir.dt.float32, tag="x")
                scratch = work_pool.tile([P, FD], mybir.dt.float32, tag="s")
                nc.vector.tensor_sub(d_tile[:, :], c_tile[:, :], a_tile[:, :])
                nc.vector.tensor_scalar_mul(x_tile[:, :], c_tile[:, :], float(NUM_BINS))
                for k in range(NK):
                    nc.vector.scalar_tensor_tensor(
                        out=scratch[:, :],
                        in0=x_tile[:, :],
                        scalar=float(k),
                        in1=d_tile[:, :],
                        op0=mybir.AluOpType.is_le,
                        op1=mybir.AluOpType.mult,
                        accum_out=acc_sb[:, j * NK + k : j * NK + k + 1],
                    )
            nc.sync.dma_start(out=out[:, :], in_=acc_sb[:, :])
    nc.compile()
    return nc


_NC_CACHE = None


def _get_nc():
    global _NC_CACHE
    if _NC_CACHE is None:
        _NC_CACHE = build_nc()
    return _NC_CACHE


def run_device(confidences, accuracies, **spmd_kwargs):
    """Run the bass kernel; returns (list of per-core partials, BassKernelResults)."""
    nc = _get_nc()
    c = np.ascontiguousarray(confidences, dtype=np.float32)
    a = np.ascontiguousarray(accuracies, dtype=np.float32)
    core_ids = list(range(N_CORES))
    in_maps = [
        {"confidences": c[i * M : (i + 1) * M], "accuracies": a[i * M : (i + 1) * M]}
        for i in core_ids
    ]
    res = run_bass_kernel_spmd(nc, in_maps, core_ids, **spmd_kwargs)
    partials = [res.results[i]["partials"] for i in core_ids]
    return partials, res


def finish(partials):
    D = np.zeros(NK, dtype=np.float64)
    for p in partials:
        D += p.reshape(P, N_TILES, NK).sum(axis=(0, 1), dtype=np.float64)
    bin_sums = D[1:] - D[:-1]  # (sum_conf - sum_acc) per bin b = 0..14
    return np.asarray(np.sum(np.abs(bin_sums)) / N, dtype=np.float32)


def kernel(confidences, accuracies, num_bins):
    assert int(num_bins) == NUM_BINS
    partials, _ = run_device(confidences, accuracies)
    return finish(partials)


# revision 2
# speedup vs baseline: 6741.0206x; 6741.0206x over previous
"""ECE (confidence calibration) kernel for 8 Trainium2 NeuronCores.

Math: reference computes idx = ceil(15*c)-1 per element (valid for c in
(0,1]), then ece = (1/N) * sum_b |sum_conf[b] - sum_acc[b]|. Using the
cumulative weighted histogram
    D_k = sum_i (c_i - a_i) * [c_i <= c*_k],        k = 1..15,
where c*_k = max{float32 c : fl(15*c) <= k} (precomputed, exhaustively
verified equivalent to the reference's fp32 binning; c*_15 = 1.0 exactly so
the c <= 1 validity bound is also exact), each bin's (sum_conf - sum_acc) is
D_{k+1} - D_k, so
    ece = (1/N) * sum_{b=0}^{14} |D_{b+1} - D_b|,   D_0 := 0.
Elements with c <= 0 would need a D_0 term (= -#{c==0, a==1}); dropping it
shifts ece by <= #such/N ~ 1e-7 relative — far below the fp32 accumulation
noise of the reference itself (~5e-4).

Mapping: data-parallel over 8 cores (2^21 elements each). Per core, 8 tiles
of [128 x 2048] f32: one DVE tensor_sub for d = c - a, then 15 fused
scalar_tensor_tensor passes ((c is_le c*_k) * d with free-dim accumulate) —
one per threshold — writing per-(tile,k) partial sums. Host sums partials in
float64 and finishes the 15-element ece. DVE-bound at ~290 us/core; DMA
(16 MiB/core) fully hidden.
"""
import numpy as np
import concourse.bacc as bacc
import concourse.mybir as mybir
from concourse.tile import TileContext
from concourse.bass_utils import run_bass_kernel_spmd

N = 16777216
NUM_BINS = 15
N_CORES = 8
P = 128
FD = 2048
M = N // N_CORES  # per-core elements
N_TILES = M // (P * FD)
F32 = mybir.dt.float32
A = mybir.AluOpType


def _cstar_thresholds(num_bins=NUM_BINS):
    """c*_k = max float32 c with fl(c*num_bins) <= k, k = 1..num_bins."""
    out = []
    for k in range(1, num_bins + 1):
        lo_u = np.array(0.0, np.float32).view(np.uint32).item()
        hi_u = np.array(2.0, np.float32).view(np.uint32).item()
        while hi_u - lo_u > 1:
            mid_u = (hi_u + lo_u) // 2
            mid = np.array(mid_u, np.uint32).view(np.float32)
            if np.float32(mid * np.float32(num_bins)) <= np.float32(k):
                lo_u = mid_u
            else:
                hi_u = mid_u
        out.append(np.array(lo_u, np.uint32).view(np.float32).item())
    return out


CSTAR = _cstar_thresholds()


def build_nc(repeat=1):
    nc = bacc.Bacc(None)
    conf = nc.dram_tensor("confidences", [M], F32, kind="ExternalInput")
    acc_in = nc.dram_tensor("accuracies", [M], F32, kind="ExternalInput")
    out = nc.dram_tensor(
        "partials", [P, N_TILES * NUM_BINS], F32, kind="ExternalOutput"
    )
    conf_t = conf.rearrange("(n p f) -> n p f", p=P, f=FD)
    acc_t = acc_in.rearrange("(n p f) -> n p f", p=P, f=FD)

    with TileContext(nc) as tc:
        with (
            tc.tile_pool(name="io", bufs=3) as io_pool,
            tc.tile_pool(name="work", bufs=2) as work_pool,
            tc.tile_pool(name="accp", bufs=1) as acc_pool,
        ):
            acc_sb = acc_pool.tile([P, N_TILES * NUM_BINS], F32, name="acc_sb")
            for j in [jj for _ in range(repeat) for jj in range(N_TILES)]:
                c_tile = io_pool.tile([P, FD], F32, tag="c", name="c_tile")
                a_tile = io_pool.tile([P, FD], F32, tag="a", name="a_tile")
                nc.sync.dma_start(out=c_tile[:, :], in_=conf_t[j, :, :])
                nc.sync.dma_start(out=a_tile[:, :], in_=acc_t[j, :, :])
                d_tile = work_pool.tile([P, FD], F32, tag="d", name="d_tile")
                s_dve = work_pool.tile([P, FD], F32, tag="s", name="s_dve", bufs=1)
                nc.vector.tensor_sub(d_tile[:, :], c_tile[:, :], a_tile[:, :])
                for i in range(NUM_BINS):
                    nc.vector.scalar_tensor_tensor(
                        out=s_dve[:, :],
                        in0=c_tile[:, :],
                        scalar=CSTAR[i],
                        in1=d_tile[:, :],
                        op0=A.is_le,
                        op1=A.mult,
                        accum_out=acc_sb[:, j * NUM_BINS + i : j * NUM_BINS + i + 1],
                    )
            nc.sync.dma_start(out=out[:, :], in_=acc_sb[:, :])
    nc.compile()
    return nc


_NC_CACHE = None


def _get_nc():
    global _NC_CACHE
    if _NC_CACHE is None:
        _NC_CACHE = build_nc()
    return _NC_CACHE


def run_device(confidences, accuracies, **spmd_kwargs):
    nc = _get_nc()
    c = np.ascontiguousarray(confidences, dtype=np.float32)
    a = np.ascontiguousarray(accuracies, dtype=np.float32)
    core_ids = list(range(N_CORES))
    in_maps = [
        {"confidences": c[i * M : (i + 1) * M], "accuracies": a[i * M : (i + 1) * M]}
        for i in core_ids
    ]
    res = run_bass_kernel_spmd(nc, in_maps, core_ids, **spmd_kwargs)
    partials = [res.results[i]["partials"] for i in core_ids]
    return partials, res


def finish(partials):
    Dk = np.zeros(NUM_BINS, dtype=np.float64)
    for p in partials:
        Dk += p.reshape(P, N_TILES, NUM_BINS).sum(axis=(0, 1), dtype=np.float64)
    D = np.concatenate([[0.0], Dk])  # D_0 = 0 (see module docstring)
    return np.asarray(np.sum(np.abs(D[1:] - D[:-1])) / N, dtype=np.float32)


def kernel(confidences, accuracies, num_bins):
    assert int(num_bins) == NUM_BINS
    partials, _ = run_device(confidences, accuracies)
    return finish(partials)


# revision 3
# speedup vs baseline: 7746.4379x; 1.1491x over previous
"""ECE (confidence calibration) kernel for 8 Trainium2 NeuronCores.

Math: reference bins by idx = ceil(15*c)-1 for valid c in (0,1], then
ece = (1/N) * sum_b |sum_conf[b] - sum_acc[b]|.

Hybrid two-engine evaluation (DVE was the bottleneck; Act engine absorbs the
top three bins):
- Bins 0..11 (DVE): cumulative weighted histogram
      D_k = sum_i (c_i - a_i) * [c_i <= c*_k],  k = 1..12,
  one fused scalar_tensor_tensor (compare+mult+free-dim-accumulate) per k,
  where c*_k = max{float32 c : fl(15c) <= k} (host-precomputed, exhaustively
  verified to reproduce the reference's fp32 binning; c*_15 = 1.0 so the
  c <= 1 validity bound is exact). Bin b delta = D_{b+1} - D_b; elements with
  c <= 0 cancel in differences, c > 1 appear nowhere.
- Bins 12..14 (Act engine, runs in parallel): relu/sign moments with fused
  accumulate, at k = 12..15:
      R_k     = sum relu(15c - k)
      SIGN_k  = sum sign(15c - k)         -> cntgt_k = (SIGN_k + N)/2
      SIGNY_k = sum sign(y - (2 + c*_k))  -> acnt_k  = (SIGNY_k + N)/2
  with y = 2a + c (a=0 gives y < 1, always sign -1). Then per bin b:
      sum_conf[b] = ((R_b - R_{b+1}) + b*cnt_b - cntgt_{b+1}) / 15
      sum_acc[b]  = acnt_b - acnt_{b+1},   cnt_b = cntgt_b - cntgt_{b+1}.
  Exactness caveats, all orders below the reference's own fp32 accumulation
  noise (~5e-4): sign(0)=0 half-counts the handful of exact-boundary
  elements (~7 in 2^24); fl(2a+c) rounds c at 2^-22 near bin edges (~30
  elements); x/15 vs c rounding ~1e-10; dropped D_0 term (= -#{c==0,a==1})
  ~1e-7.

Mapping: data-parallel over 8 cores (2^21 elements each), 8 tiles of
[128 x 2048] f32 per core; host sums per-(core,tile) partials in float64 and
finishes the 15-element ece. Measured ~257 us on-device (DVE 14 passes/tile,
Act 12 passes/tile overlapped; 16 MiB/core DMA fully hidden).
"""
import numpy as np
import concourse.bacc as bacc
import concourse.mybir as mybir
from concourse.tile import TileContext
from concourse.bass_utils import run_bass_kernel_spmd

N = 16777216
NUM_BINS = 15
N_CORES = 8
P = 128
FD = 2048
M = N // N_CORES
N_TILES = M // (P * FD)
F32 = mybir.dt.float32
A = mybir.AluOpType
ACT = mybir.ActivationFunctionType

B = 12                                # bins 0..11 on DVE, 12..14 on Act
KCNT = list(range(B, NUM_BINS + 1))   # count-side thresholds k = 12..15
NCOL = B + 3 * len(KCNT)              # accum columns per tile (12 + 12)


def _cstar_thresholds(num_bins=NUM_BINS):
    """c*_k = max float32 c with fl(c*num_bins) <= k, k = 1..num_bins."""
    out = []
    for k in range(1, num_bins + 1):
        lo_u = np.array(0.0, np.float32).view(np.uint32).item()
        hi_u = np.array(2.0, np.float32).view(np.uint32).item()
        while hi_u - lo_u > 1:
            mid_u = (hi_u + lo_u) // 2
            mid = np.array(mid_u, np.uint32).view(np.float32)
            if np.float32(mid * np.float32(num_bins)) <= np.float32(k):
                lo_u = mid_u
            else:
                hi_u = mid_u
        out.append(np.array(lo_u, np.uint32).view(np.float32).item())
    return out


CSTAR = _cstar_thresholds()


def build_nc(repeat=1):
    nc = bacc.Bacc(None)
    conf = nc.dram_tensor("confidences", [M], F32, kind="ExternalInput")
    acc_in = nc.dram_tensor("accuracies", [M], F32, kind="ExternalInput")
    out = nc.dram_tensor("partials", [P, N_TILES * NCOL], F32,
                         kind="ExternalOutput")
    conf_t = conf.rearrange("(n p f) -> n p f", p=P, f=FD)
    acc_t = acc_in.rearrange("(n p f) -> n p f", p=P, f=FD)

    with TileContext(nc) as tc:
        with (
            tc.tile_pool(name="io", bufs=3) as io_pool,
            tc.tile_pool(name="work", bufs=2) as work_pool,
            tc.tile_pool(name="accp", bufs=1) as acc_pool,
        ):
            acc_sb = acc_pool.tile([P, N_TILES * NCOL], F32, name="acc_sb")
            bias_sb = acc_pool.tile([P, 8], F32, name="bias_sb")
            for i, k in enumerate(KCNT):
                nc.vector.memset(bias_sb[:, i : i + 1], float(-k))
                t = float(np.float32(2.0) + np.float32(CSTAR[k - 1]))
                nc.vector.memset(bias_sb[:, 4 + i : 5 + i], -t)
            for j in [jj for _ in range(repeat) for jj in range(N_TILES)]:
                c_tile = io_pool.tile([P, FD], F32, tag="c", name="c_tile")
                a_tile = io_pool.tile([P, FD], F32, tag="a", name="a_tile")
                nc.sync.dma_start(out=c_tile[:, :], in_=conf_t[j, :, :])
                nc.sync.dma_start(out=a_tile[:, :], in_=acc_t[j, :, :])
                d_tile = work_pool.tile([P, FD], F32, tag="d", name="d_tile")
                y_tile = work_pool.tile([P, FD], F32, tag="y", name="y_tile")
                s_dve = work_pool.tile([P, FD], F32, tag="s", name="s_dve", bufs=1)
                s_act = work_pool.tile([P, FD], F32, tag="sa", name="s_act", bufs=1)
                base = j * NCOL
                nc.vector.tensor_sub(d_tile[:, :], c_tile[:, :], a_tile[:, :])
                nc.vector.scalar_tensor_tensor(
                    out=y_tile[:, :], in0=a_tile[:, :], scalar=2.0,
                    in1=c_tile[:, :], op0=A.mult, op1=A.add)
                for i in range(B):
                    nc.vector.scalar_tensor_tensor(
                        out=s_dve[:, :], in0=c_tile[:, :], scalar=CSTAR[i],
                        in1=d_tile[:, :], op0=A.is_le, op1=A.mult,
                        accum_out=acc_sb[:, base + i : base + i + 1])
                for i in range(len(KCNT)):
                    nc.scalar.activation(
                        s_act[:, :], c_tile[:, :], ACT.Relu,
                        bias=bias_sb[:, i : i + 1], scale=15.0,
                        accum_out=acc_sb[:, base + B + i : base + B + i + 1])
                    nc.scalar.activation(
                        s_act[:, :], c_tile[:, :], ACT.Sign,
                        bias=bias_sb[:, i : i + 1], scale=15.0,
                        accum_out=acc_sb[:, base + B + 4 + i : base + B + 5 + i])
                    nc.scalar.activation(
                        s_act[:, :], y_tile[:, :], ACT.Sign,
                        bias=bias_sb[:, 4 + i : 5 + i], scale=1.0,
                        accum_out=acc_sb[:, base + B + 8 + i : base + B + 9 + i])
            nc.sync.dma_start(out=out[:, :], in_=acc_sb[:, :])
    nc.compile()
    return nc


_NC_CACHE = None


def _get_nc():
    global _NC_CACHE
    if _NC_CACHE is None:
        _NC_CACHE = build_nc()
    return _NC_CACHE


def run_device(confidences, accuracies, **spmd_kwargs):
    nc = _get_nc()
    c = np.ascontiguousarray(confidences, dtype=np.float32)
    a = np.ascontiguousarray(accuracies, dtype=np.float32)
    core_ids = list(range(N_CORES))
    in_maps = [
        {"confidences": c[i * M : (i + 1) * M], "accuracies": a[i * M : (i + 1) * M]}
        for i in core_ids
    ]
    res = run_bass_kernel_spmd(nc, in_maps, core_ids, **spmd_kwargs)
    partials = [res.results[i]["partials"] for i in core_ids]
    return partials, res


def finish(partials):
    agg = np.zeros(NCOL, dtype=np.float64)
    for p in partials:
        agg += p.reshape(P, N_TILES, NCOL).sum(axis=(0, 1), dtype=np.float64)
    D = np.concatenate([[0.0], agg[:B]])          # D_0..D_12
    R = agg[B : B + 4]
    cntgt = (agg[B + 4 : B + 8] + N) / 2.0        # k = 12..15
    acnt = (agg[B + 8 : B + 12] + N) / 2.0
    deltas = list(D[1:] - D[:-1])                 # bins 0..11
    for bi, b in enumerate(range(B, NUM_BINS)):   # bins 12..14
        cnt_b = cntgt[bi] - cntgt[bi + 1]
        sum_conf = ((R[bi] - R[bi + 1]) + b * cnt_b - cntgt[bi + 1]) / 15.0
        sum_acc = acnt[bi] - acnt[bi + 1]
        deltas.append(sum_conf - sum_acc)
    return np.asarray(np.sum(np.abs(np.array(deltas))) / N, dtype=np.float32)


def kernel(confidences, accuracies, num_bins):
    assert int(num_bins) == NUM_BINS
    partials, _ = run_device(confidences, accuracies)
    return finish(partials)
